# revision 13
# baseline (speedup 1.0000x reference)
# Trainium2 Bass kernel for the (buggy-faithful) single-head self-attention:
#   WQ  = q @ Wq ; WKV = q @ Wv  (Wv used for both K and V, as in reference)
#   S   = (WQ @ WKV^T) / sqrt(D) ; S[mask==1] = -1e9 ; P = softmax(S) ; out = P @ WKV
#
# Sharding: 8 cores = 4 batches x 2 query-halves. Each core holds the full
# K/V (from its batch) and computes a [1024, 1024] slice of the output.
# The key axis is column-permuted per core (own query block first) so the
# SPMD program can address "my query rows" at a fixed offset.
from contextlib import ExitStack

import numpy as np

import concourse.bass as bass
import concourse.mybir as mybir
import concourse.tile as tile
from concourse import bacc
from concourse.bass_utils import run_bass_kernel_spmd
from concourse.masks import make_identity

F32 = mybir.dt.float32
F32R = mybir.dt.float32r
BF16 = mybir.dt.bfloat16
I32 = mybir.dt.int32

P = 128  # partitions
CH = 512  # moving-operand chunk (fp32 max, = 1 PSUM bank)


def build_nc(
    S_full=2048,
    S_core=1024,
    D=1024,
    n_cores=8,
    use_f32r=True,
    act_copies=True,
    reps=1,
):
    """Build the per-core Bass program (same program on every core)."""
    NQ = S_core // P  # query row-tiles per core
    NK = S_full // P  # key tiles
    NCK = S_full // CH  # key chunks (MM2 rhs)
    NCQ = S_core // CH  # query chunks (MM1 rhs = first NCQ chunks)
    ND = D // P  # contraction tiles for MM1/MM2
    NE = D // P  # e (output-feature) tiles
    NEH = NE // 2  # e-tiles per half
    DH = D // 2
    # NOTE: 1/sqrt(D) score scaling is folded into Wq on the host.
    mask_mul = -1.0e9
    QDT = F32R if use_f32r else F32

    nc = bacc.Bacc(
        "TRN2", target_bir_lowering=False, debug=False, num_devices=n_cores
    )
    qp_d = nc.dram_tensor("qp", [NCK, P, ND, CH], QDT, kind="ExternalInput").ap()
    wq_d = nc.dram_tensor("wq", [2, P, ND, DH], QDT, kind="ExternalInput").ap()
    wv_d = nc.dram_tensor("wv", [2, P, ND, DH], QDT, kind="ExternalInput").ap()
    mk_d = nc.dram_tensor("mask", [NQ, P, S_full], I32, kind="ExternalInput").ap()
    id_d = nc.dram_tensor("ident", [P, P], QDT, kind="ExternalInput").ap()
    out_d = nc.dram_tensor("out", [NQ, P, D], F32, kind="ExternalOutput").ap()

    with tile.TileContext(nc) as tc:
      for _rep in range(reps):
        stack = ExitStack()
        # ---- long-lived SBUF ----
        consts = stack.enter_context(tc.tile_pool(name="consts", bufs=1))
        wqt_pool = stack.enter_context(tc.tile_pool(name="wqt", bufs=1))
        wkvt_pool = stack.enter_context(tc.tile_pool(name="wkvt", bufs=1))
        wkv_pool = stack.enter_context(tc.tile_pool(name="wkv", bufs=1))

        id_f32 = consts.tile([P, P], QDT, tag="idf")
        nc.sync.dma_start(out=id_f32[:, :], in_=id_d[:, :])
        id_bf16 = consts.tile([P, P], BF16, tag="idb")
        make_identity(nc, id_bf16[:, :])

        wqt = wqt_pool.tile([P, NE, S_core], QDT)  # WQ^T  [e, sq]
        wkvt = wkvt_pool.tile([P, NE, S_full], QDT)  # WKV^T [e, k]
        wkv = wkv_pool.tile([P, NK, D], BF16)  # WKV   [k, e]

        # ---- prologue: MM1 (WQ^T) + MM2 (WKV^T) + WKV^T->WKV transposes ----
        with (
            tc.tile_pool(name="qstream", bufs=2) as qpool,
            tc.tile_pool(name="wstream", bufs=1) as wpool,
            tc.tile_pool(name="ppsum", bufs=8, space="PSUM") as pp,
        ):

            def wkv_transposes(trange, groups):
                # 4 PE transposes into one [P, 4*P] PSUM tile, then a single
                # DVE copy into the bf16 natural-layout WKV tile.
                for t in trange:
                    for g in groups:
                        pst = pp.tile([P, CH], QDT, tag="mm")
                        for u in range(4):
                            e = g * 4 + u
                            nc.tensor.transpose(
                                pst[:, u * P : (u + 1) * P],
                                wkvt[:, e, t * P : (t + 1) * P],
                                id_f32[:, :],
                            )
                        nc.vector.tensor_copy(
                            wkv[:, t, g * 4 * P : (g + 1) * 4 * P], pst[:, :]
                        )

            hd = ND // 2
            for half in range(2):
                wqh = wpool.tile([P, ND, DH], QDT, tag="wqh")
                wvh = wpool.tile([P, ND, DH], QDT, tag="wvh")
                qt0 = None
                if half == 0:
                    # order the first loads so the first matmul chain can
                    # start after ~2MB instead of the full 6MB
                    nc.sync.dma_start(out=wvh[:, :hd, :], in_=wv_d[0, :, :hd, :])
                    qt0 = qpool.tile([P, ND, CH], QDT, tag="qt")
                    nc.sync.dma_start(out=qt0[:, :hd, :], in_=qp_d[0, :, :hd, :])
                    nc.sync.dma_start(out=wvh[:, hd:, :], in_=wv_d[0, :, hd:, :])
                    nc.sync.dma_start(out=qt0[:, hd:, :], in_=qp_d[0, :, hd:, :])
                    nc.sync.dma_start(out=wqh[:, :hd, :], in_=wq_d[0, :, :hd, :])
                    nc.sync.dma_start(out=wqh[:, hd:, :], in_=wq_d[0, :, hd:, :])
                else:
                    nc.sync.dma_start(out=wvh[:, :hd, :], in_=wv_d[1, :, :hd, :])
                    nc.sync.dma_start(out=wvh[:, hd:, :], in_=wv_d[1, :, hd:, :])
                    nc.sync.dma_start(out=wqh[:, :hd, :], in_=wq_d[1, :, :hd, :])
                    nc.sync.dma_start(out=wqh[:, hd:, :], in_=wq_d[1, :, hd:, :])
                for ck in range(NCK):
                    if half == 0 and ck == 0:
                        qt = qt0
                    else:
                        qt = qpool.tile([P, ND, CH], QDT, tag="qt")
                        nc.sync.dma_start(out=qt[:, :hd, :], in_=qp_d[ck, :, :hd, :])
                        nc.sync.dma_start(out=qt[:, hd:, :], in_=qp_d[ck, :, hd:, :])
                    for et in range(NEH):
                        e = half * NEH + et
                        es = slice(et * P, (et + 1) * P)
                        ps_kv = pp.tile([P, CH], F32, tag="mm")
                        for j in range(ND):
                            nc.tensor.matmul(
                                ps_kv[:, :],
                                wvh[:, j, es],
                                qt[:, j, :],
                                start=(j == 0),
                                stop=(j == ND - 1),
                            )
                        # wkvt copies on ACT to keep DVE free for the
                        # transpose copies that gate PE
                        kv_dst = wkvt[:, e, ck * CH : (ck + 1) * CH]
                        if act_copies:
                            nc.scalar.copy(kv_dst, ps_kv[:, :])
                        else:
                            nc.vector.tensor_copy(kv_dst, ps_kv[:, :])
                        if ck < NCQ:
                            ps_q = pp.tile([P, CH], F32, tag="mm")
                            for j in range(ND):
                                nc.tensor.matmul(
                                    ps_q[:, :],
                                    wqh[:, j, es],
                                    qt[:, j, :],
                                    start=(j == 0),
                                    stop=(j == ND - 1),
                                )
                            nc.vector.tensor_copy(
                                wqt[:, e, ck * CH : (ck + 1) * CH], ps_q[:, :]
                            )
                    if half == 1:
                        # overlap remaining WKV transposes with the MM stream;
                        # the needed wkvt key-columns complete with this ck
                        span = NK // NCK
                        wkv_transposes(
                            range(ck * span, (ck + 1) * span),
                            range(NEH // 4, NE // 4),
                        )
                if half == 0:
                    # e-tiles of half 0 are final: transpose them now; their
                    # DVE copies overlap the half-1 weight/q DMA reload
                    wkv_transposes(range(NK), range(NEH // 4))

        # ---- main loop over query tiles (MM3 pipelined one ahead) ----
        with (
            tc.tile_pool(name="mk", bufs=2) as mk_pool,
            tc.tile_pool(name="mf", bufs=2) as mf_pool,
            tc.tile_pool(name="ssb", bufs=2) as ssb_pool,
            tc.tile_pool(name="pp16", bufs=2) as p_pool,
            tc.tile_pool(name="pt16", bufs=2) as pt_pool,
            tc.tile_pool(name="ob", bufs=2) as ob_pool,
            tc.tile_pool(name="scal", bufs=2) as sc_pool,
            tc.tile_pool(name="scps", bufs=2, space="PSUM") as scps,
            tc.tile_pool(name="trps", bufs=2, space="PSUM") as trps,
            tc.tile_pool(name="ops", bufs=1, space="PSUM") as ops,
        ):
            mfs = [None] * NQ

            def load_mask(i):
                mk = mk_pool.tile([P, S_full], I32, tag="mk")
                nc.sync.dma_start(out=mk[:, :], in_=mk_d[i])
                mfs[i] = mf_pool.tile([P, S_full], F32, tag="mf", name=f"mf{i}")
                nc.vector.tensor_scalar_mul(mfs[i][:, :], mk[:, :], float(mask_mul))

            def mm3(i):
                halves = []
                nck_h = NCK // 2
                for h2 in range(2):
                    sch = scps.tile(
                        [P, S_full // 2], F32, tag="sc", name=f"sc{i}_{h2}"
                    )
                    for e in range(NE):
                        for c2 in range(nck_h):
                            ck = h2 * nck_h + c2
                            nc.tensor.matmul(
                                sch[:, c2 * CH : (c2 + 1) * CH],
                                wqt[:, e, i * P : (i + 1) * P],
                                wkvt[:, e, ck * CH : (ck + 1) * CH],
                                start=(e == 0),
                                stop=(e == NE - 1),
                            )
                    halves.append(sch)
                return halves

            def softmax(i, sc):
                ssb = ssb_pool.tile([P, S_full], F32, tag="ssb")
                sh = S_full // 2
                nc.vector.tensor_add(ssb[:, :sh], sc[0][:, :], mfs[i][:, :sh])
                nc.vector.tensor_add(ssb[:, sh:], sc[1][:, :], mfs[i][:, sh:])
                ngm = sc_pool.tile([P, 1], F32, tag="ngm")
                nc.vector.tensor_reduce(
                    ngm[:, :],
                    ssb[:, :],
                    axis=mybir.AxisListType.X,
                    op=mybir.AluOpType.max,
                    negate=True,
                )
                pb = p_pool.tile([P, S_full], BF16, tag="pb")
                sm = sc_pool.tile([P, 1], F32, tag="sm")
                nc.scalar.activation(
                    pb[:, :],
                    ssb[:, :],
                    mybir.ActivationFunctionType.Exp,
                    bias=ngm[:, 0:1],
                    scale=1.0,
                    accum_out=sm[:, 0:1],
                )
                rin = sc_pool.tile([P, 1], F32, tag="rin")
                nc.vector.reciprocal(rin[:, :], sm[:, :])
                return pb, rin

            def pv_and_store(i, pb, rin):
                # transpose the P row-block in 4-column-tile groups,
                # interleaved with the accumulating P^T @ WKV matmuls
                ptt = pt_pool.tile([P, NK * P], BF16, tag="ptt")
                op = ops.tile([P, D], F32, tag="op")
                for g in range(NK // 4):
                    tr = trps.tile([P, 4 * P], BF16, tag="tr16")
                    for u in range(4):
                        t = g * 4 + u
                        nc.tensor.transpose(
                            tr[:, u * P : (u + 1) * P],
                            pb[:, t * P : (t + 1) * P],
                            id_bf16[:, :],
                        )
                    nc.vector.tensor_copy(
                        ptt[:, g * 4 * P : (g + 1) * 4 * P], tr[:, :]
                    )
                    for u in range(4):
                        t = g * 4 + u
                        for ck in range(D // CH):
                            nc.tensor.matmul(
                                op[:, ck * CH : (ck + 1) * CH],
                                ptt[:, t * P : (t + 1) * P],
                                wkv[:, t, ck * CH : (ck + 1) * CH],
                                start=(t == 0),
                                stop=(t == NK - 1),
                            )
                ob = ob_pool.tile([P, D], F32, tag="ob")
                nc.scalar.activation(
                    ob[:, :],
                    op[:, :],
                    mybir.ActivationFunctionType.Copy,
                    bias=0.0,
                    scale=rin[:, 0:1],
                )
                nc.sync.dma_start(out=out_d[i], in_=ob[:, :])

            load_mask(0)
            sc_cur = mm3(0)
            prev = None  # (i, pb, rin)
            for i in range(NQ):
                if prev is not None:
                    pv_and_store(*prev)
                pb, rin = softmax(i, sc_cur)
                if i + 1 < NQ:
                    load_mask(i + 1)
                    sc_cur = mm3(i + 1)
                prev = (i, pb, rin)
            pv_and_store(*prev)
        stack.close()

    nc.compile()
    return nc


def _pack_w(w, D):
    # [D, D] -> [2, 128, D//128, D//2]; value = w[j*128+p, half*D/2 + c]
    return np.ascontiguousarray(w.reshape(D // P, P, 2, D // 2).transpose(2, 1, 0, 3))


def make_in_maps(q, Wq, Wv, mask, S_full=2048, S_core=1024, D=1024, n_cores=8):
    B = q.shape[0]
    # fold the 1/sqrt(D) attention scaling into Wq
    wqp = _pack_w(np.asarray(Wq, np.float32) / np.float32(np.sqrt(D)), D)
    wvp = _pack_w(np.asarray(Wv, np.float32), D)
    NCK = S_full // CH
    ident = np.eye(P, dtype=np.float32)
    in_maps = []
    for c in range(n_cores):
        b, h = divmod(c, n_cores // B)
        ro = slice(h * S_core, (h + 1) * S_core)
        rt_lo = 0 if h == 1 else S_core
        rt = slice(rt_lo, rt_lo + (S_full - S_core))
        q_r = np.concatenate([q[b][ro], q[b][rt]], axis=0)  # [S_full, D] perm rows
        qp = np.ascontiguousarray(
            q_r.reshape(NCK, CH, D // P, P).transpose(0, 3, 2, 1)
        )
        mrows = mask[ro]
        mk = np.ascontiguousarray(
            np.concatenate([mrows[:, ro], mrows[:, rt]], axis=1)
            .astype(np.int32)
            .reshape(S_core // P, P, S_full)
        )
        in_maps.append(
            {"qp": qp, "wq": wqp, "wv": wvp, "mask": mk, "ident": ident}
        )
    return in_maps


_CACHE = {}


def _get_nc():
    if "nc" not in _CACHE:
        _CACHE["nc"] = build_nc()
    return _CACHE["nc"]


def kernel(q, Wq, Wk, Wv, mask):
    q = np.asarray(q, np.float32)
    Wq = np.asarray(Wq, np.float32)
    Wv = np.asarray(Wv, np.float32)  # Wk unused: reference applies Wv for K and V
    mask = np.asarray(mask, np.int32)
    B, S, D_ = q.shape
    S_core = 1024
    nc = _get_nc()
    in_maps = make_in_maps(q, Wq, Wv, mask)
    res = run_bass_kernel_spmd(nc, in_maps, list(range(8)))
    _CACHE["last_result"] = res
    out = np.empty((B, S, D_), np.float32)
    for c in range(8):
        b, h = divmod(c, 2)
        out[b, h * S_core : (h + 1) * S_core] = res.results[c]["out"].reshape(
            S_core, D_
        )
    return out


# revision 17
# speedup vs baseline: 1329.2969x; 1329.2969x over previous
# Trainium2 Bass kernel for the (buggy-faithful) single-head self-attention:
#   WQ  = q @ Wq ; WKV = q @ Wv  (Wv used for both K and V, as in reference)
#   S   = (WQ @ WKV^T) / sqrt(D) ; S[mask==1] = -1e9 ; P = softmax(S) ; out = P @ WKV
#
# Sharding: 8 cores = 4 batches x 2 query-halves. Each core holds the full
# K/V (from its batch) and computes a [1024, 1024] slice of the output.
# The key axis is column-permuted per core (own query block first) so the
# SPMD program can address "my query rows" at a fixed offset.
from contextlib import ExitStack

import numpy as np

import concourse.bass as bass
import concourse.mybir as mybir
import concourse.tile as tile
from concourse import bacc
from concourse.bass_utils import run_bass_kernel_spmd
from concourse.masks import make_identity

F32 = mybir.dt.float32
F32R = mybir.dt.float32r
BF16 = mybir.dt.bfloat16
I32 = mybir.dt.int32

P = 128  # partitions
CH = 512  # moving-operand chunk (fp32 max, = 1 PSUM bank)


def build_nc(
    S_full=2048,
    S_core=1024,
    D=1024,
    n_cores=8,
    use_f32r=True,
    act_copies=True,
    reps=1,
):
    """Build the per-core Bass program (same program on every core)."""
    NQ = S_core // P  # query row-tiles per core
    NK = S_full // P  # key tiles
    NCK = S_full // CH  # key chunks (MM2 rhs)
    NCQ = S_core // CH  # query chunks (MM1 rhs = first NCQ chunks)
    ND = D // P  # contraction tiles for MM1/MM2
    NE = D // P  # e (output-feature) tiles
    NEH = NE // 2  # e-tiles per half
    DH = D // 2
    # NOTE: 1/sqrt(D) score scaling is folded into Wq on the host.
    mask_mul = -1.0e9
    QDT = F32R if use_f32r else F32

    nc = bacc.Bacc(
        "TRN2", target_bir_lowering=False, debug=False, num_devices=n_cores
    )
    qp_d = nc.dram_tensor("qp", [NCK, P, ND, CH], QDT, kind="ExternalInput").ap()
    wq_d = nc.dram_tensor("wq", [2, P, ND, DH], QDT, kind="ExternalInput").ap()
    wv_d = nc.dram_tensor("wv", [2, P, ND, DH], QDT, kind="ExternalInput").ap()
    mk_d = nc.dram_tensor("mask", [NQ, P, S_full], I32, kind="ExternalInput").ap()
    id_d = nc.dram_tensor("ident", [P, P], QDT, kind="ExternalInput").ap()
    out_d = nc.dram_tensor("out", [NQ, P, D], F32, kind="ExternalOutput").ap()

    with tile.TileContext(nc) as tc:
      for _rep in range(reps):
        stack = ExitStack()
        # ---- long-lived SBUF ----
        consts = stack.enter_context(tc.tile_pool(name="consts", bufs=1))
        wqt_pool = stack.enter_context(tc.tile_pool(name="wqt", bufs=1))
        wkvt_pool = stack.enter_context(tc.tile_pool(name="wkvt", bufs=1))
        wkv_pool = stack.enter_context(tc.tile_pool(name="wkv", bufs=1))

        id_f32 = consts.tile([P, P], QDT, tag="idf")
        nc.sync.dma_start(out=id_f32[:, :], in_=id_d[:, :])
        id_bf16 = consts.tile([P, P], BF16, tag="idb")
        make_identity(nc, id_bf16[:, :])

        wqt = wqt_pool.tile([P, NE, S_core], QDT)  # WQ^T  [e, sq]
        wkvt = wkvt_pool.tile([P, NE, S_full], QDT)  # WKV^T [e, k]
        wkv = wkv_pool.tile([P, NK, D], BF16)  # WKV   [k, e]

        # ---- prologue: MM1 (WQ^T) + MM2 (WKV^T) + WKV^T->WKV transposes ----
        with (
            tc.tile_pool(name="qstream", bufs=2) as qpool,
            tc.tile_pool(name="wstream", bufs=1) as wpool,
            tc.tile_pool(name="ppsum", bufs=8, space="PSUM") as pp,
        ):

            def wkv_transposes(trange, groups):
                # 4 PE transposes into one [P, 4*P] PSUM tile, then a single
                # DVE copy into the bf16 natural-layout WKV tile.
                for t in trange:
                    for g in groups:
                        pst = pp.tile([P, CH], QDT, tag="mm")
                        for u in range(4):
                            e = g * 4 + u
                            nc.tensor.transpose(
                                pst[:, u * P : (u + 1) * P],
                                wkvt[:, e, t * P : (t + 1) * P],
                                id_f32[:, :],
                            )
                        nc.vector.tensor_copy(
                            wkv[:, t, g * 4 * P : (g + 1) * 4 * P], pst[:, :]
                        )

            hd = ND // 2
            for half in range(2):
                wqh = wpool.tile([P, ND, DH], QDT, tag="wqh")
                wvh = wpool.tile([P, ND, DH], QDT, tag="wvh")
                qt0 = None
                if half == 0:
                    # order the first loads so the first matmul chain can
                    # start after ~2MB instead of the full 6MB
                    nc.sync.dma_start(out=wvh[:, :hd, :], in_=wv_d[0, :, :hd, :])
                    qt0 = qpool.tile([P, ND, CH], QDT, tag="qt")
                    nc.sync.dma_start(out=qt0[:, :hd, :], in_=qp_d[0, :, :hd, :])
                    nc.sync.dma_start(out=wvh[:, hd:, :], in_=wv_d[0, :, hd:, :])
                    nc.sync.dma_start(out=qt0[:, hd:, :], in_=qp_d[0, :, hd:, :])
                    nc.sync.dma_start(out=wqh[:, :hd, :], in_=wq_d[0, :, :hd, :])
                    nc.sync.dma_start(out=wqh[:, hd:, :], in_=wq_d[0, :, hd:, :])
                else:
                    nc.sync.dma_start(out=wvh[:, :hd, :], in_=wv_d[1, :, :hd, :])
                    nc.sync.dma_start(out=wvh[:, hd:, :], in_=wv_d[1, :, hd:, :])
                    nc.sync.dma_start(out=wqh[:, :hd, :], in_=wq_d[1, :, :hd, :])
                    nc.sync.dma_start(out=wqh[:, hd:, :], in_=wq_d[1, :, hd:, :])
                for ck in range(NCK):
                    if half == 0 and ck == 0:
                        qt = qt0
                    else:
                        qt = qpool.tile([P, ND, CH], QDT, tag="qt")
                        nc.sync.dma_start(out=qt[:, :hd, :], in_=qp_d[ck, :, :hd, :])
                        nc.sync.dma_start(out=qt[:, hd:, :], in_=qp_d[ck, :, hd:, :])
                    for et in range(NEH):
                        e = half * NEH + et
                        es = slice(et * P, (et + 1) * P)
                        ps_kv = pp.tile([P, CH], F32, tag="mm")
                        for j in range(ND):
                            nc.tensor.matmul(
                                ps_kv[:, :],
                                wvh[:, j, es],
                                qt[:, j, :],
                                start=(j == 0),
                                stop=(j == ND - 1),
                            )
                        # wkvt copies on ACT to keep DVE free for the
                        # transpose copies that gate PE
                        kv_dst = wkvt[:, e, ck * CH : (ck + 1) * CH]
                        if act_copies:
                            nc.scalar.copy(kv_dst, ps_kv[:, :])
                        else:
                            nc.vector.tensor_copy(kv_dst, ps_kv[:, :])
                        if ck < NCQ:
                            ps_q = pp.tile([P, CH], F32, tag="mm")
                            for j in range(ND):
                                nc.tensor.matmul(
                                    ps_q[:, :],
                                    wqh[:, j, es],
                                    qt[:, j, :],
                                    start=(j == 0),
                                    stop=(j == ND - 1),
                                )
                            nc.vector.tensor_copy(
                                wqt[:, e, ck * CH : (ck + 1) * CH], ps_q[:, :]
                            )
                    if half == 1:
                        # overlap remaining WKV transposes with the MM stream;
                        # the needed wkvt key-columns complete with this ck
                        span = NK // NCK
                        wkv_transposes(
                            range(ck * span, (ck + 1) * span),
                            range(NEH // 4, NE // 4),
                        )
                if half == 0:
                    # e-tiles of half 0 are final: transpose them now; their
                    # DVE copies overlap the half-1 weight/q DMA reload
                    wkv_transposes(range(NK), range(NEH // 4))

        # ---- main loop over query tiles (MM3 pipelined one ahead) ----
        with (
            tc.tile_pool(name="mk", bufs=2) as mk_pool,
            tc.tile_pool(name="mf", bufs=2) as mf_pool,
            tc.tile_pool(name="ssb", bufs=2) as ssb_pool,
            tc.tile_pool(name="pp16", bufs=2) as p_pool,
            tc.tile_pool(name="pt16", bufs=2) as pt_pool,
            tc.tile_pool(name="ob", bufs=2) as ob_pool,
            tc.tile_pool(name="scal", bufs=2) as sc_pool,
            tc.tile_pool(name="scps", bufs=2, space="PSUM") as scps,
            tc.tile_pool(name="trps", bufs=2, space="PSUM") as trps,
            tc.tile_pool(name="ops", bufs=1, space="PSUM") as ops,
        ):
            mfs = [None] * NQ

            def load_mask(i):
                mk = mk_pool.tile([P, S_full], I32, tag="mk")
                nc.sync.dma_start(out=mk[:, :], in_=mk_d[i])
                mfs[i] = mf_pool.tile([P, S_full], F32, tag="mf", name=f"mf{i}")
                nc.vector.tensor_scalar_mul(mfs[i][:, :], mk[:, :], float(mask_mul))

            def mm3(i):
                halves = []
                nck_h = NCK // 2
                for h2 in range(2):
                    sch = scps.tile(
                        [P, S_full // 2], F32, tag="sc", name=f"sc{i}_{h2}"
                    )
                    for e in range(NE):
                        for c2 in range(nck_h):
                            ck = h2 * nck_h + c2
                            nc.tensor.matmul(
                                sch[:, c2 * CH : (c2 + 1) * CH],
                                wqt[:, e, i * P : (i + 1) * P],
                                wkvt[:, e, ck * CH : (ck + 1) * CH],
                                start=(e == 0),
                                stop=(e == NE - 1),
                            )
                    halves.append(sch)
                return halves

            def softmax(i, sc):
                ssb = ssb_pool.tile([P, S_full], F32, tag="ssb")
                sh = S_full // 2
                nc.vector.tensor_add(ssb[:, :sh], sc[0][:, :], mfs[i][:, :sh])
                nc.vector.tensor_add(ssb[:, sh:], sc[1][:, :], mfs[i][:, sh:])
                ngm = sc_pool.tile([P, 1], F32, tag="ngm")
                nc.vector.tensor_reduce(
                    ngm[:, :],
                    ssb[:, :],
                    axis=mybir.AxisListType.X,
                    op=mybir.AluOpType.max,
                    negate=True,
                )
                pb = p_pool.tile([P, S_full], BF16, tag="pb")
                sm = sc_pool.tile([P, 1], F32, tag="sm")
                nc.scalar.activation(
                    pb[:, :],
                    ssb[:, :],
                    mybir.ActivationFunctionType.Exp,
                    bias=ngm[:, 0:1],
                    scale=1.0,
                    accum_out=sm[:, 0:1],
                )
                rin = sc_pool.tile([P, 1], F32, tag="rin")
                nc.vector.reciprocal(rin[:, :], sm[:, :])
                return pb, rin

            def pv_and_store(i, pb, rin):
                # transpose the P row-block in 4-column-tile groups,
                # interleaved with the accumulating P^T @ WKV matmuls
                ptt = pt_pool.tile([P, NK * P], BF16, tag="ptt")
                op = ops.tile([P, D], F32, tag="op")
                for g in range(NK // 4):
                    tr = trps.tile([P, 4 * P], BF16, tag="tr16")
                    for u in range(4):
                        t = g * 4 + u
                        nc.tensor.transpose(
                            tr[:, u * P : (u + 1) * P],
                            pb[:, t * P : (t + 1) * P],
                            id_bf16[:, :],
                        )
                    nc.vector.tensor_copy(
                        ptt[:, g * 4 * P : (g + 1) * 4 * P], tr[:, :]
                    )
                    for u in range(4):
                        t = g * 4 + u
                        for ck in range(D // CH):
                            nc.tensor.matmul(
                                op[:, ck * CH : (ck + 1) * CH],
                                ptt[:, t * P : (t + 1) * P],
                                wkv[:, t, ck * CH : (ck + 1) * CH],
                                start=(t == 0),
                                stop=(t == NK - 1),
                            )
                ob = ob_pool.tile([P, D], F32, tag="ob")
                nc.scalar.activation(
                    ob[:, :],
                    op[:, :],
                    mybir.ActivationFunctionType.Copy,
                    bias=0.0,
                    scale=rin[:, 0:1],
                )
                nc.sync.dma_start(out=out_d[i], in_=ob[:, :])

            load_mask(0)
            sc_cur = mm3(0)
            prev = None  # (i, pb, rin)
            for i in range(NQ):
                if prev is not None:
                    pv_and_store(*prev)
                pb, rin = softmax(i, sc_cur)
                if i + 1 < NQ:
                    load_mask(i + 1)
                    sc_cur = mm3(i + 1)
                prev = (i, pb, rin)
            pv_and_store(*prev)
        stack.close()

    nc.compile()
    return nc


def _pack_w(w, D):
    # [D, D] -> [2, 128, D//128, D//2]; value = w[j*128+p, half*D/2 + c]
    return np.ascontiguousarray(w.reshape(D // P, P, 2, D // 2).transpose(2, 1, 0, 3))


def make_in_maps(q, Wq, Wv, mask, S_full=2048, S_core=1024, D=1024, n_cores=8):
    B = q.shape[0]
    # fold the 1/sqrt(D) attention scaling into Wq
    wqp = _pack_w(np.asarray(Wq, np.float32) / np.float32(np.sqrt(D)), D)
    wvp = _pack_w(np.asarray(Wv, np.float32), D)
    NCK = S_full // CH
    ident = np.eye(P, dtype=np.float32)
    in_maps = []
    for c in range(n_cores):
        b, h = divmod(c, n_cores // B)
        ro = slice(h * S_core, (h + 1) * S_core)
        rt_lo = 0 if h == 1 else S_core
        rt = slice(rt_lo, rt_lo + (S_full - S_core))
        q_r = np.concatenate([q[b][ro], q[b][rt]], axis=0)  # [S_full, D] perm rows
        qp = np.ascontiguousarray(
            q_r.reshape(NCK, CH, D // P, P).transpose(0, 3, 2, 1)
        )
        mrows = mask[ro]
        mk = np.ascontiguousarray(
            np.concatenate([mrows[:, ro], mrows[:, rt]], axis=1)
            .astype(np.int32)
            .reshape(S_core // P, P, S_full)
        )
        in_maps.append(
            {"qp": qp, "wq": wqp, "wv": wvp, "mask": mk, "ident": ident}
        )
    return in_maps


_CACHE = {}


def _get_nc():
    if "nc" not in _CACHE:
        _CACHE["nc"] = build_nc()
    return _CACHE["nc"]


def kernel(q, Wq, Wk, Wv, mask):
    q = np.asarray(q, np.float32)
    Wq = np.asarray(Wq, np.float32)
    Wv = np.asarray(Wv, np.float32)  # Wk unused: reference applies Wv for K and V
    mask = np.asarray(mask, np.int32)
    B, S, D_ = q.shape
    S_core = 1024
    nc = _get_nc()
    in_maps = make_in_maps(q, Wq, Wv, mask)
    res = run_bass_kernel_spmd(nc, in_maps, list(range(8)))
    _CACHE["last_result"] = res
    out = np.empty((B, S, D_), np.float32)
    for c in range(8):
        b, h = divmod(c, 2)
        out[b, h * S_core : (h + 1) * S_core] = res.results[c]["out"].reshape(
            S_core, D_
        )
    return out
